# revision 40
# baseline (speedup 1.0000x reference)
"""Contrastive loss kernel for Trainium2 (8 NeuronCores, Bass/Tile).

Strategy
--------
Only rows with label==1 (pos) contribute losses, and only columns with
label==0 (neg) plus the diagonal enter each row's logsumexp.  The host
computes the tiny index sets from `labels`, then each of the 8 cores
(2 per batch) receives its half of the batch's positive rows and all of
the batch's negative english rows, padded to uniform shapes (P1, N1).

Device pipeline (all reference FLOPs on device; host only gathers /
pads / transposes / casts):
  - g arrives twice: raw fp8e4 (x8) transposed [h%128, ktile, row] as
    the DoubleRow matmul stationary, and bf16 rows for norms + diag
    dots.  g's normalization folds into the exp's per-partition scale
    SG_p = 1/(64*T*|g_p|), so g is never scaled on device.
  - e-neg rows arrive bf16; per 128-row chunk: sum-of-squares (DVE
    stt+accum; DVE is the only engine walrus allows accumulators on),
    rsqrt via shared Ln/Exp pairs (ACT; fold constants pre-applied at
    the squares), diag(s_q) built by an affine_select from a broadcast
    (GPSIMD), then one regular matmul per h-half computes
    transpose-and-scale fused: out[h, q] = e[q, h] * s_q.  PSUM ->
    SBUF fp8 casts on DVE/ACT (GPSIMD cannot touch PSUM on hardware).
  - Main matmuls run in fp8e4 DoubleRow perf mode: one matmul per
    512-col slab contracts all H=256 (2 k-tiles) at 0.5 cycles/row.
  - exp(logit*SG_p - 15) runs in-place on the PSUM tile with accum_out
    giving per-row negative sums S.  Chunk 0 is exp'd per sub-slab so
    ACT starts as soon as the first e-chunk is through the pipe.
    Padded columns give exactly exp(-15), removed by a correction.
  - diag: bf16 row dots (DVE, accum) scaled by both rsqrt norms; the
    per-row loss is ln(exp(diag-15)+S+corr)+15-diag, masked and
    row-reduced; host sums the 8x128 partials and divides by count.
All ACT-table ops (Ln/Exp) interleave with the exp stream explicitly:
each engine queue executes in emission order, so every ACT op is
emitted at the point its inputs are expected to be ready.
"""

import sys

if "/opt/trn_rl_repo" not in sys.path:
    sys.path.insert(0, "/opt/trn_rl_repo")

from contextlib import ExitStack

import ml_dtypes
import numpy as np

import concourse.bass as bass
import concourse.tile as tile
from concourse import mybir
from concourse.bass_utils import run_bass_kernel_spmd
from concourse.masks import make_identity

TEMPERATURE = 0.07
IGNORE_INDEX = -100
CMAX = 15.0
H = 256
N_CORES = 8
FP8_SCALE = 8.0

LAST_RESULTS = None
LAST_SHAPES = None
TRACE = False


def _legalize_waits(nc: bass.Bass, max_waits: int = 1) -> None:
    """This container's walrus accepts at most one sync-wait per instruction
    (ACT structs especially); Tile can emit several.  Split the excess onto
    same-engine NoOps placed immediately before the instruction."""
    for bb in nc.main_func.blocks:
        new = []
        for ins in bb.instructions:
            si = ins.sync_info
            if si is not None and si.on_wait and len(si.on_wait) > max_waits:
                waits = list(si.on_wait)
                extra, keep = waits[:-max_waits], waits[-max_waits:]
                for i in range(0, len(extra), max_waits):
                    new.append(
                        mybir.InstNoOp(
                            name=nc.get_next_instruction_name(),
                            engine=ins.engine,
                            ins=[],
                            outs=[],
                            sync_info=mybir.SyncInfo(
                                on_wait=extra[i : i + max_waits], on_update=[]
                            ),
                            bass_nofuse=True,
                        )
                    )
                ins.sync_info = mybir.SyncInfo(
                    on_wait=keep, on_update=list(si.on_update or [])
                )
            new.append(ins)
        bb.instructions[:] = new


def _build_program(P1: int, N1: int, legalize: bool = True) -> bass.Bass:
    PC = P1 // 128
    N1r = ((N1 + 127) // 128) * 128  # row-layout / transpose granularity
    NC = N1r // 128
    f32 = mybir.dt.float32
    bf16 = mybir.dt.bfloat16
    fp8 = mybir.dt.float8e4
    OP = mybir.AluOpType
    AF = mybir.ActivationFunctionType
    DR = mybir.MatmulPerfMode.DoubleRow

    nc = bass.Bass()
    g8t = nc.dram_tensor("g8t", [128, 2, P1], fp8, kind="ExternalInput")
    gb = nc.dram_tensor("gb", [128, PC, H], bf16, kind="ExternalInput")
    ep = nc.dram_tensor("ep", [128, PC, H], bf16, kind="ExternalInput")
    en = nc.dram_tensor("en", [128, NC, H], bf16, kind="ExternalInput")
    wv = nc.dram_tensor("wv", [128, PC], f32, kind="ExternalInput")
    corr = nc.dram_tensor("corr", [1, 1], f32, kind="ExternalInput")
    out = nc.dram_tensor("out", [128, 2], f32, kind="ExternalOutput")

    # exp blocks for chunk 0: sub-slab boundaries (bank-safe: within-bank or
    # bank-aligned).  [0:128] starts as early as possible.
    blk0 = [0, 128, 512, 1024, N1]
    blk0 = [b for b in blk0 if b < N1] + [N1]
    NB = len(blk0) - 1
    # e-chunk membership of each block
    blk_chunks = [list(range(blk0[i] // 128, (blk0[i + 1] + 127) // 128)) for i in range(NB)]
    # full-width matmul slabs for chunks >= 1
    slabs = [(s, min(s + 512, N1)) for s in range(0, N1, 512)]

    with tile.TileContext(nc) as tc, ExitStack() as ctx:
        persist = ctx.enter_context(tc.tile_pool(name="persist", bufs=1))
        small = ctx.enter_context(tc.tile_pool(name="small", bufs=1))
        scratch = ctx.enter_context(tc.tile_pool(name="scratch", bufs=4))
        dgpool = ctx.enter_context(tc.tile_pool(name="dgpool", bufs=3))
        psum_tp = ctx.enter_context(tc.tile_pool(name="psum_tp", bufs=2, space="PSUM"))
        psum_mm = ctx.enter_context(tc.tile_pool(name="psum_mm", bufs=2, space="PSUM"))

        # ---- constants
        eps_t = small.tile([128, 1], f32)
        nc.gpsimd.memset(eps_t[:], 1e-24)
        cneg_t = small.tile([128, 1], f32)
        nc.gpsimd.memset(cneg_t[:], -CMAX)

        # ---- DMAs.  scalar queue: small first pieces then bulk; ACT's own
        # compute (dummy table load) is emitted after so it runs during the
        # DVE/GPSIMD square phase.  sync queue: e-chunk 0 first.
        EnA = persist.tile([128, 1, H], bf16)     # e chunk 0
        EnB = persist.tile([128, 3, H], bf16)     # e chunks 1-3
        EnC = persist.tile([128, 4, H], bf16, name="EnC") if NC > 4 else None
        EnD = persist.tile([128, NC - 8, H], bf16, name="EnD") if NC > 8 else None
        Gb0 = persist.tile([128, 1, H], bf16)     # g chunk 0
        GbR = persist.tile([128, PC - 1, H], bf16, name="GbR") if PC > 1 else None
        nc.scalar.dma_start(out=Gb0[:], in_=gb[:, 0:1, :])
        if EnC is not None:
            nc.scalar.dma_start(out=EnC[:], in_=en[:, 4:8, :])
        # ACT table preload after the two small scalar-queue DMAs; all other
        # DMAs go on sync so ACT's SEQ is free for table ops.
        dummy = small.tile([128, 1], f32)
        nc.scalar.activation(
            out=dummy[:], in_=eps_t[:], func=AF.Ln, bias=eps_t[:, 0:1], scale=1.0
        )
        nc.sync.dma_start(out=EnA[:], in_=en[:, 0:1, :])
        nc.sync.dma_start(out=EnB[:], in_=en[:, 1:4, :])
        if GbR is not None:
            nc.sync.dma_start(out=GbR[:], in_=gb[:, 1:PC, :])
        G8 = persist.tile([128, 2, P1], fp8)
        nc.sync.dma_start(out=G8[:], in_=g8t[:])
        if EnD is not None:
            nc.sync.dma_start(out=EnD[:], in_=en[:, 8:NC, :])
        Ef = persist.tile([128, PC, H], bf16)
        nc.sync.dma_start(out=Ef[:], in_=ep[:])
        wt = small.tile([128, PC], f32)
        nc.sync.dma_start(out=wt[:], in_=wv[:])
        corr_t = small.tile([128, 1], f32)
        nc.sync.dma_start(out=corr_t[:], in_=corr[:].to_broadcast([128, 1]))

        ident = small.tile([128, 128], bf16)
        make_identity(nc, ident[:])

        # Norm tiles.  Pre-scaling at the square folds the rsqrt constants so
        # one shared Ln/Exp pair serves a whole segment group:
        #   e:  accum = ssn/64            -> rsqrt = 8*rsqrt(ssn) = s_q
        #   g:  accum = ssg*(64T)^2       -> rsqrt = SG = rsqrt(ssg)/(64T)
        #   ep: accum = sse/4096          -> rsqrt = 64*rsqrt(sse)
        SC_E = 1.0 / (FP8_SCALE * FP8_SCALE)
        SC_G = (FP8_SCALE * FP8_SCALE * TEMPERATURE) ** 2
        SC_P = 1.0 / (FP8_SCALE * FP8_SCALE) ** 2
        NB4 = NC - 4  # e-chunks >= 4
        nsA0 = small.tile([128, 1], f32)                # e0
        nsA = small.tile([128, 5], f32)                 # e1-3 (cols 1-3), g0
        nsB = small.tile([128, max(1, NB4 + PC - 1)], f32)  # e4.., g1..
        nsC = small.tile([128, PC], f32)                # ep
        eT8 = persist.tile([128, 2, N1r], fp8)

        def sn(c):
            if c == 0:
                return nsA0[:, 0:1]
            return nsA[:, c : c + 1] if c < 4 else nsB[:, c - 4 : c - 3]

        def sg(c):
            return nsA[:, 4:5] if c == 0 else nsB[:, NB4 + c - 1 : NB4 + c]

        def en_src(c):
            if c == 0:
                return EnA[:, 0, :]
            if c < 4:
                return EnB[:, c - 1, :]
            if c < 8:
                return EnC[:, c - 4, :]
            return EnD[:, c - 8, :]

        def square_acc(eng, src, acc, scale=1.0):
            sq = scratch.tile([128, H], bf16, tag="sq")
            eng.scalar_tensor_tensor(
                out=sq[:], in0=src, scalar=scale, in1=src,
                op0=OP.mult, op1=OP.mult, accum_out=acc,
            )

        def rsqrt_of(ss):
            nc.scalar.activation(out=ss, in_=ss, func=AF.Ln, bias=eps_t[:, 0:1], scale=1.0)
            nc.scalar.activation(out=ss, in_=ss, func=AF.Exp, bias=0.0, scale=-0.5)

        def diag_build(c):
            dg = dgpool.tile([128, 128], bf16, tag="dg")
            nc.gpsimd.affine_select(
                out=dg[:],
                in_=sn(c).to_broadcast([128, 128]),
                compare_op=OP.is_equal,
                fill=0.0, base=0, pattern=[[-1, 128]], channel_multiplier=1,
            )
            return dg

        def diag_build_dve(c):
            dg = dgpool.tile([128, 128], bf16, tag="dg")
            nc.vector.tensor_scalar_mul(dg[:], ident[:], sn(c))
            return dg

        def transp_copy_pair(cs, copy_eng):
            # cs: consecutive chunks sharing one 1-bank psum tile laid out
            # [hk, chunk, 128] so one contiguous copy serves both.
            pt = psum_tp.tile([128, 2, len(cs), 128], f32, tag="pt")
            for i, c in enumerate(cs):
                dg = diag_build(c)
                for hk in range(2):
                    nc.tensor.matmul(
                        pt[:, hk, i, :],
                        en_src(c)[:, hk * 128 : (hk + 1) * 128],
                        dg[:],
                        start=True, stop=True,
                    )
            c0 = cs[0]
            dst = eT8[:, :, c0 * 128 : (c0 + len(cs)) * 128]
            if copy_eng is nc.scalar:
                nc.scalar.copy(out=dst, in_=pt[:])
            else:
                copy_eng.tensor_copy(out=dst, in_=pt[:])

        def transp_copy(c, copy_eng, dve_diag=False):
            dg = diag_build_dve(c) if dve_diag else diag_build(c)
            pt = psum_tp.tile([128, 2, 128], f32, tag="pt")
            for hk in range(2):
                nc.tensor.matmul(
                    pt[:, hk, :], en_src(c)[:, hk * 128 : (hk + 1) * 128], dg[:],
                    start=True, stop=True,
                )
            dst = eT8[:, :, c * 128 : (c + 1) * 128]
            if copy_eng is nc.scalar:
                nc.scalar.copy(out=dst, in_=pt[:])
            else:
                copy_eng.tensor_copy(out=dst, in_=pt[:])

        pm_cols = 512 * ((N1 * 4 + 2047) // 2048)
        pm_tiles = {}

        def new_pm(c):
            pm_tiles[c] = psum_mm.tile([128, pm_cols], f32, tag="pm", name=f"pm{c}")

        def main_mm(c, lo, hi):
            nc.tensor.matmul(
                pm_tiles[c][:, lo:hi],
                G8[:, :, c * 128 : (c + 1) * 128],
                eT8[:, :, lo:hi],
                start=True, stop=True, perf_mode=DR,
            )

        # chunk-0 block partials in S0; chunks >=1 in SR[c-1]; summed at tail
        S0 = small.tile([128, NB + 1], f32)
        SR = small.tile([128, PC], f32)

        def exp_acc(c, lo, hi, s_col):
            acc = S0[:, s_col : s_col + 1] if c == 0 else SR[:, c : c + 1]
            nc.scalar.activation(
                out=pm_tiles[c][:, lo:hi], in_=pm_tiles[c][:, lo:hi], func=AF.Exp,
                bias=cneg_t[:, 0:1], scale=sg(c),
                accum_out=acc,
            )

        # ================= emission =================
        # squares chunk0-3 + g0 (DVE: e0,e1; GPSIMD: e2,e3,g0)
        square_acc(nc.vector, en_src(0), nsA0[:, 0:1], SC_E)
        rsqrt_of(nsA0[:])
        square_acc(nc.vector, Gb0[:, 0, :], nsA[:, 4:5], SC_G)
        square_acc(nc.vector, en_src(1), nsA[:, 1:2], SC_E)
        for c in range(2, 4):
            square_acc(nc.vector, en_src(c), nsA[:, c : c + 1], SC_E)
        rsqrt_of(nsA[:, 1:5])
        # chunk 0 pipe -> first tiny exp block [0:128]
        transp_copy(0, nc.scalar)
        new_pm(0)
        main_mm(0, blk0[0], blk0[1])
        exp_acc(0, blk0[0], blk0[1], 0)
        # e4-7 squares + their rsqrt BEFORE the chunk1-3 copies so the pair
        # wins ACT priority (it gates diag4-7 -> the copy pipeline).
        for c in range(4, min(8, NC)):
            square_acc(nc.vector, en_src(c), nsB[:, c - 4 : c - 3], SC_E)
        transp_copy(1, nc.scalar)
        rsqrt_of(nsB[:, 0 : min(4, NB4)])
        # cp2/3 AFTER the whole pair so the Exp isn't wedged out by priority
        transp_copy_pair([2, 3], nc.scalar)
        main_mm(0, blk0[1], blk0[2])
        exp_acc(0, blk0[1], blk0[2], 1)

        if NC > 4:
            h8 = min(8, NC)
            for c0 in range(4, h8, 2):
                if c0 + 1 < h8:
                    transp_copy_pair([c0, c0 + 1], nc.vector)
                else:
                    transp_copy(c0, nc.vector)
            if NB > 2:
                main_mm(0, blk0[2], blk0[3])
                exp_acc(0, blk0[2], blk0[3], 2)
        # e8-10 pipe + chunk-0 last block
        for c in range(8, NC):
            square_acc(nc.vector, en_src(c), nsB[:, c - 4 : c - 3], SC_E)
        if NB4 > 4:
            rsqrt_of(nsB[:, 4:NB4])
        if NC > 8:
            for c0 in range(8, NC, 2):
                if c0 + 1 < NC:
                    transp_copy_pair([c0, c0 + 1], nc.vector)
                else:
                    transp_copy(c0, nc.vector)
        if NB > 3:
            main_mm(0, blk0[3], blk0[4])
            exp_acc(0, blk0[3], blk0[4], 3)
        # g chunk 1 early (gates exp1); chunks 2+ deferred below
        if PC > 1:
            square_acc(nc.vector, GbR[:, 0, :], nsB[:, NB4 : NB4 + 1], SC_G)
            rsqrt_of(nsB[:, NB4 : NB4 + 1])
        if PC > 2:
            for c in range(2, PC):
                square_acc(nc.vector, GbR[:, c - 1, :], nsB[:, NB4 + c - 1 : NB4 + c], SC_G)
            rsqrt_of(nsB[:, NB4 + 1 : NB4 + PC - 1])
        if NC > 4:
            new_pm(1)
            main_mm(1, 0, 512)
            nc.scalar.activation(
                out=pm_tiles[1][:, 0:512], in_=pm_tiles[1][:, 0:512], func=AF.Exp,
                bias=cneg_t[:, 0:1], scale=sg(1),
                accum_out=S0[:, NB : NB + 1],
            )

        # diag-path squares/dots on DVE (run during the exp phase)
        dgots = small.tile([128, PC], f32)
        for c in range(PC):
            gsrc = Gb0[:, 0, :] if c == 0 else GbR[:, c - 1, :]
            dsq = scratch.tile([128, H], bf16, tag="dsq")
            nc.vector.scalar_tensor_tensor(
                out=dsq[:], in0=gsrc, scalar=1.0, in1=Ef[:, c, :],
                op0=OP.mult, op1=OP.mult, accum_out=dgots[:, c : c + 1],
            )
        for c in range(PC):
            square_acc(nc.vector, Ef[:, c, :], nsC[:, c : c + 1], SC_P)
        rsqrt_of(nsC[:])

        # chunk-0 partial folds into SR[:, 0] (DVE smalls, hidden)
        nc.vector.tensor_tensor(
            out=SR[:, 0:1], in0=S0[:, 0:1], in1=S0[:, 1:2], op=OP.add
        )
        for i in range(2, NB):
            nc.vector.tensor_tensor(
                out=SR[:, 0:1], in0=SR[:, 0:1], in1=S0[:, i : i + 1], op=OP.add
            )
        # chunks 1..: full-width matmuls + one exp each; diag/ed slotted in
        diag = small.tile([128, PC], f32)
        ed = small.tile([128, PC], f32)
        t2 = small.tile([128, PC], f32)
        loss = small.tile([128, PC], f32)
        lm = small.tile([128, PC], f32)
        part = small.tile([128, 2], f32)
        for c in range(1, PC):
            if c not in pm_tiles:
                new_pm(c)
            for lo, hi in slabs:
                if c == 1 and lo == 0:
                    continue
                main_mm(c, lo, hi)
            if c == 2 and NC > 4:
                nc.vector.tensor_tensor(
                    out=SR[:, 1:2], in0=SR[:, 1:2], in1=S0[:, NB : NB + 1], op=OP.add
                )
            if c == 3 or (c == PC - 1 and PC <= 3):
                nc.vector.tensor_mul(diag[:, 0:1], dgots[:, 0:1], nsA[:, 4:5])
                if PC > 1:
                    nc.vector.tensor_mul(
                        diag[:, 1:PC], dgots[:, 1:PC], nsB[:, NB4 : NB4 + PC - 1]
                    )
                nc.vector.tensor_mul(diag[:], diag[:], nsC[:])
                nc.scalar.activation(
                    out=ed[:], in_=diag[:], func=AF.Exp, bias=cneg_t[:, 0:1], scale=1.0
                )
            if c == PC - 1:
                # chunks 0..PC-2 loss tail slots before the final exp
                nc.vector.scalar_tensor_tensor(
                    out=t2[:, 0 : PC - 1], in0=SR[:, 0 : PC - 1],
                    scalar=corr_t[:, 0:1], in1=ed[:, 0 : PC - 1],
                    op0=OP.add, op1=OP.add,
                )
                nc.scalar.activation(
                    out=t2[:, 0 : PC - 1], in_=t2[:, 0 : PC - 1], func=AF.Ln,
                    bias=eps_t[:, 0:1], scale=1.0,
                )
                nc.vector.scalar_tensor_tensor(
                    out=loss[:, 0 : PC - 1], in0=t2[:, 0 : PC - 1], scalar=CMAX,
                    in1=diag[:, 0 : PC - 1], op0=OP.add, op1=OP.subtract,
                )
                nc.vector.scalar_tensor_tensor(
                    out=lm[:, 0 : PC - 1], in0=loss[:, 0 : PC - 1], scalar=1.0,
                    in1=wt[:, 0 : PC - 1], op0=OP.mult, op1=OP.mult,
                    accum_out=part[:, 0:1],
                )
            if c == 1 and NC > 4:
                exp_acc(c, 512, N1, c)
            else:
                exp_acc(c, 0, N1, c)

        PCa = PC - 1
        # last chunk after the final exp
        nc.vector.scalar_tensor_tensor(
            out=t2[:, PCa:PC], in0=SR[:, PCa:PC], scalar=corr_t[:, 0:1], in1=ed[:, PCa:PC],
            op0=OP.add, op1=OP.add,
        )
        nc.scalar.activation(
            out=t2[:, PCa:PC], in_=t2[:, PCa:PC], func=AF.Ln, bias=eps_t[:, 0:1], scale=1.0
        )
        nc.vector.scalar_tensor_tensor(
            out=loss[:, PCa:PC], in0=t2[:, PCa:PC], scalar=CMAX, in1=diag[:, PCa:PC],
            op0=OP.add, op1=OP.subtract,
        )
        nc.vector.scalar_tensor_tensor(
            out=lm[:, PCa:PC], in0=loss[:, PCa:PC], scalar=1.0, in1=wt[:, PCa:PC],
            op0=OP.mult, op1=OP.mult, accum_out=part[:, 1:2],
        )
        nc.sync.dma_start(out=out[:], in_=part[:])
    if legalize:
        _legalize_waits(nc, max_waits=1)
    return nc


def _pad_rows(x: np.ndarray, n: int) -> np.ndarray:
    outp = np.zeros((n,) + x.shape[1:], dtype=x.dtype)
    outp[: x.shape[0]] = x
    return outp


def kernel(greek_embeds, english_embeds, labels):
    global LAST_RESULTS, LAST_SHAPES
    g = np.ascontiguousarray(np.asarray(greek_embeds, dtype=np.float32))
    e = np.ascontiguousarray(np.asarray(english_embeds, dtype=np.float32))
    lab = np.asarray(labels)
    B, P, Hh = g.shape
    assert Hh == H and B * 2 == N_CORES

    valid = lab != IGNORE_INDEX
    pos = valid & (lab == 1)
    neg = valid & (lab != 1)
    ok = (valid.sum(-1) >= 2) & pos.any(-1) & neg.any(-1)

    count = int(pos[ok].sum()) if ok.any() else 0
    if count == 0:
        return np.float32(0.0)

    pos_idx = [np.nonzero(pos[b])[0] if ok[b] else np.zeros(0, np.int64) for b in range(B)]
    neg_idx = [np.nonzero(neg[b])[0] if ok[b] else np.zeros(0, np.int64) for b in range(B)]
    halves = [np.array_split(pi, 2) for pi in pos_idx]

    np_max = max(len(halves[b][h]) for b in range(B) for h in range(2))
    nn_max = max(len(ni) for ni in neg_idx)
    P1 = max(128, ((np_max + 127) // 128) * 128)
    N1 = max(512, nn_max)          # exact exp/matmul width
    N1r = ((N1 + 127) // 128) * 128
    PC, NC = P1 // 128, N1r // 128

    E15 = np.float32(np.exp(np.float32(-CMAX)))
    bf16 = ml_dtypes.bfloat16
    fp8 = ml_dtypes.float8_e4m3
    in_maps = []
    for core in range(N_CORES):
        b, hf = core // 2, core % 2
        p_idx = halves[b][hf]
        n_idx = neg_idx[b]
        gr = _pad_rows(g[b][p_idx], P1)
        er = _pad_rows(e[b][n_idx], N1r)
        epr = _pad_rows(e[b][p_idx], P1)
        w = np.zeros((128, PC), np.float32)
        npos = len(p_idx)
        for c in range(PC):
            w[: max(0, min(128, npos - c * 128)), c] = 1.0
        in_maps.append(
            {
                "g8t": np.ascontiguousarray(
                    (gr * FP8_SCALE).reshape(P1, 2, 128).transpose(2, 1, 0)
                ).astype(fp8),
                "gb": np.ascontiguousarray(
                    gr.astype(bf16).reshape(PC, 128, H).transpose(1, 0, 2)
                ),
                "ep": np.ascontiguousarray(
                    epr.astype(bf16).reshape(PC, 128, H).transpose(1, 0, 2)
                ),
                "en": np.ascontiguousarray(
                    er.astype(bf16).reshape(NC, 128, H).transpose(1, 0, 2)
                ),
                "wv": w,
                "corr": np.array([[-(N1 - len(n_idx)) * float(E15)]], np.float32),
            }
        )

    LAST_SHAPES = (P1, N1, dict(in_maps[0]))
    nc = _build_program(P1, N1)
    res = run_bass_kernel_spmd(nc, in_maps, list(range(N_CORES)), trace=TRACE)
    LAST_RESULTS = res
    total = sum(float(r["out"].sum()) for r in res.results)
    return np.float32(total / count)
